# revision 5
# baseline (speedup 1.0000x reference)
"""Trainium2 Bass kernel for nn_ConstGCN.

Math note: in the reference, the attention score s[b,i] is constant along
the softmax axis j, and softmax is shift-invariant, so
p = softmax(s + mask) = softmax(mask) and p.sum(axis=2) == 1 (to ~1e-6 in
f32).  The output therefore collapses to

    out = relu(text + mean_k(emb_table[const_labels[...,k]]) @ fc_W.T + fc_b)

which depends on neither const_mat nor attn_W/attn_b.  The embedding + fc
further fuse into a single gather table M2 = (emb_table @ fc_W.T)/8, so

    out[b,l,:] = relu(text[b,l,:] + sum_k M2[labels[b,l,k], :] + fc_b)

On device (per core, data-parallel over batch: 2 of 16 batches = 4096
positions):
  - one-hot counts over the 100 label classes are built with a bf16
    is_equal against a replicated iota constant, then reduced over K
    (DVE);  the bias folds in as two padded labels of class 100 with
    M2[100] = fc_b/2
  - PE transposes counts to [class, position] via identity matmul
  - PE accumulates I @ text + counts.T @ M2 in PSUM
  - ACT applies relu, DMA streams out
const_mat (256 MiB) is never read.
"""

import numpy as np
import ml_dtypes

B, L, D = 16, 2048, 256
CN, K = 100, 8
KP = 10            # K padded with two bias slots (class CN, M2 row CN = fc_b/2)
NCLS = CN + 1      # 101 classes incl. bias class
NCORES = 8
POS = (B // NCORES) * L          # 4096 positions per core
CHUNK = 512                      # positions per chunk
NCHUNK = POS // CHUNK            # 8
Q = CHUNK // 128                 # 4 position-groups of 128 per chunk

_compiled = None


def _build():
    import concourse.bacc as bacc
    import concourse.mybir as mybir
    from concourse.tile import TileContext

    f32 = mybir.dt.float32
    bf16 = mybir.dt.bfloat16

    nc = bacc.Bacc("TRN2", target_bir_lowering=False)

    text_d = nc.dram_tensor("text", [POS, D], f32, kind="ExternalInput")
    lab_d = nc.dram_tensor("labels", [NCHUNK, 128, Q * KP], bf16,
                           kind="ExternalInput")
    m2_d = nc.dram_tensor("m2", [NCLS, D], f32, kind="ExternalInput")
    out_d = nc.dram_tensor("out", [POS, D], f32, kind="ExternalOutput")

    # constants embedded in the NEFF
    iota_np = np.repeat(np.arange(NCLS, dtype=np.float32), KP)  # [NCLS*KP]
    iota_np = np.broadcast_to(iota_np, (128, NCLS * KP)).astype(
        ml_dtypes.bfloat16)
    iota_d = nc.inline_tensor(np.ascontiguousarray(iota_np), name="iota")
    ident_d = nc.inline_tensor(np.eye(128, dtype=np.float32), name="ident")

    # position index within a chunk: partition p, group q  <->  p*Q + q
    text_v = text_d.rearrange("(n p q) d -> n p (q d)", p=128, q=Q)
    out_v = out_d.rearrange("(n p q) d -> n p (q d)", p=128, q=Q)

    with TileContext(nc) as tc:
        with (
            tc.tile_pool(name="const", bufs=1) as cpool,
            tc.tile_pool(name="work", bufs=2) as wpool,
            tc.tile_pool(name="io", bufs=2) as iopool,
            tc.tile_pool(name="ct", bufs=4) as ctpool,
            tc.tile_pool(name="ps_t", bufs=2, space="PSUM") as pst,
            tc.tile_pool(name="ps_a", bufs=4, space="PSUM") as psa,
        ):
            iota_sb = cpool.tile([128, NCLS * KP], bf16)
            nc.sync.dma_start(out=iota_sb[:, :], in_=iota_d[:, :])
            ident_sb = cpool.tile([128, 128], f32)
            nc.sync.dma_start(out=ident_sb[:, :], in_=ident_d[:, :])
            m2_sb = cpool.tile([NCLS, D], f32)
            nc.sync.dma_start(out=m2_sb[:, :], in_=m2_d[:, :])

            for n in range(NCHUNK):
                lab = wpool.tile([128, Q * KP], bf16, tag="lab")
                nc.sync.dma_start(out=lab[:, :], in_=lab_d[n, :, :])

                text_t = iopool.tile([128, Q * D], f32, tag="text")
                nc.sync.dma_start(out=text_t[:, :], in_=text_v[n, :, :])

                eq = wpool.tile([128, Q * NCLS * KP], bf16, tag="eq")
                eq3 = eq.rearrange("p (q c k) -> p q c k", c=NCLS, k=KP)
                nc.vector.tensor_tensor(
                    out=eq3,
                    in0=lab.rearrange("p (q k) -> p q k", k=KP)[:, :, None, :]
                        .broadcast_to([128, Q, NCLS, KP]),
                    in1=iota_sb.rearrange("p (c k) -> p c k", k=KP)[:, None, :, :]
                        .broadcast_to([128, Q, NCLS, KP]),
                    op=mybir.AluOpType.is_equal,
                )

                counts = wpool.tile([128, Q * NCLS], f32, tag="counts")
                nc.vector.reduce_sum(
                    out=counts.rearrange("p (q c) -> p q c", c=NCLS),
                    in_=eq3,
                    axis=mybir.AxisListType.X,
                )

                res = iopool.tile([128, Q * D], f32, tag="res")
                for q in range(Q):
                    ctp = pst.tile([NCLS, 128], f32, tag="ctp")
                    nc.tensor.transpose(
                        out=ctp[:, :],
                        in_=counts[:, q * NCLS:(q + 1) * NCLS],
                        identity=ident_sb[:, :],
                    )
                    ct = ctpool.tile([NCLS, 128], f32, tag="ct")
                    nc.scalar.copy(out=ct[:, :], in_=ctp[:, :])

                    acc = psa.tile([128, D], f32, tag="acc")
                    nc.tensor.matmul(
                        acc[:, :], lhsT=ident_sb[:, :],
                        rhs=text_t[:, q * D:(q + 1) * D],
                        start=True, stop=False,
                    )
                    nc.tensor.matmul(
                        acc[:, :], lhsT=ct[:, :], rhs=m2_sb[:, :],
                        start=False, stop=True,
                    )
                    nc.scalar.activation(
                        out=res[:, q * D:(q + 1) * D], in_=acc[:, :],
                        func=mybir.ActivationFunctionType.Relu,
                    )
                nc.sync.dma_start(out=out_v[n, :, :], in_=res[:, :])

    nc.finalize()
    return nc


def _get_compiled():
    global _compiled
    if _compiled is None:
        _compiled = _build()
    return _compiled


def _prep_core_inputs(text, labels_padded, m2):
    """text: [POS, D] f32, labels_padded: [POS, KP] bf16 -> in_map."""
    lab = labels_padded.reshape(NCHUNK, 128, Q, KP)  # (n, p, q, k): pos = n*CHUNK + p*Q + q
    lab = np.ascontiguousarray(lab.reshape(NCHUNK, 128, Q * KP))
    return {
        "text": np.ascontiguousarray(text),
        "labels": lab,
        "m2": m2,
    }


def kernel(text, const_mat, const_labels, emb_table, attn_W, attn_b,
           fc_W, fc_b):
    from concourse.bass_utils import run_bass_kernel_spmd

    text = np.asarray(text, dtype=np.float32)
    const_labels = np.asarray(const_labels)
    emb_table = np.asarray(emb_table, dtype=np.float32)
    fc_W = np.asarray(fc_W, dtype=np.float32)
    fc_b = np.asarray(fc_b, dtype=np.float32)

    # fused gather table: row c (c<CN) = (emb_table @ fc_W.T)[c]/8,
    # row CN = fc_b/2 (hit exactly twice via the two padded labels)
    m2 = np.empty((NCLS, D), dtype=np.float32)
    m2[:CN] = (emb_table.astype(np.float64) @ fc_W.T.astype(np.float64)
               * 0.125).astype(np.float32)
    m2[CN] = fc_b * np.float32(0.5)

    lab = const_labels.reshape(B * L, K).astype(np.int32)
    lab_p = np.full((B * L, KP), CN, dtype=np.int32)
    lab_p[:, :K] = lab
    lab_bf = lab_p.astype(ml_dtypes.bfloat16)

    text_flat = text.reshape(B * L, D)

    nc = _get_compiled()
    in_maps = []
    for c in range(NCORES):
        sl = slice(c * POS, (c + 1) * POS)
        in_maps.append(_prep_core_inputs(text_flat[sl], lab_bf[sl], m2))

    r = run_bass_kernel_spmd(nc, in_maps, core_ids=list(range(NCORES)))
    out = np.concatenate([r.results[c]["out"] for c in range(NCORES)], axis=0)
    return out.reshape(B, L, D)
